# revision 26
# baseline (speedup 1.0000x reference)
"""Trainium2 Bass kernel for AGRNN edge-MLP message passing.

Math (per edge e):
    feat = [node_feat[dst], node_feat_lang[dst], edge_spatial[e],
            node_feat_lang[src], node_feat[src]]            # [1640]
    pred[e] = relu(feat @ W1 + b1) @ W2 + b2                # [13]

Strategy (8 NeuronCores, pure edge parallelism, no collectives):
  - Host packs a combined node table [100000, 832] bf16:
        [vis(512) | lang(300) | 1.0 | zeros(19)]
    The constant-1.0 column folds b1 into W1 (dst side only).
  - Edges sharded 25000/core (padded to 25088 = 196 tiles of 128 = 49
    groups of 512).
  - Per 128-edge tile: two 128-row indirect-DMA gathers (dst rows ->
    cols 0:832, src rows -> cols 832:1664 of a [128e, 4, 1664] group
    tile).  The HW indirect DMA handles exactly one index per
    partition, so gathers are per-tile; with deep buffering the SWDGE
    generation (~1.1us/gather) and SDMA descriptor processing
    (~160GB/s on 1.7KB rows) pipeline into the dominant ~510us wall.
  - edge_spatial (host-padded to 20 cols) is DMA'd into the src-side
    tail [1644:1664); the src gather stops at col 1644 so that region
    has a single writer -- every gather then needs only one semaphore
    wait (no event-semaphore splits stalling the Pool engine).
  - PE transposes flip each 128-col chunk to feature-major (the DMA
    xbar is not used: it would contend with the gathers on the SDMA
    engines and Tile serializes xbar-mode transitions); 4 chunks land
    in one PSUM tile and are copied out 512 wide, alternating DVE/ACT.
  - 13 accumulating bf16 matmuls (N=256) per tile produce h in PSUM;
    ACT applies relu; PE transposes h for the W2 matmuls; b2 is added
    during the DVE output copy via a precomputed broadcast.  Output
    f32 [25088, 13] per core; host trims + concatenates.

Measured (single NeuronCore == full-kernel time; no cross-core deps):
  ~578us HW exec (device-state noise up to +15%), rel err 3.3e-3,
  PE array and gather chain both ~85% busy.  Floors: gather SDMA
  ~500us (2x25088 descriptors, ~190ns engine-time each over 16
  engines), PE ~507us -- the kernel sits within ~10% of both.
"""

import os

import numpy as np
import ml_dtypes

import concourse.bass as bass
import concourse.mybir as mybir
from concourse import bacc
from concourse.tile import TileContext
from concourse.masks import make_identity

BF16 = mybir.dt.bfloat16
F32 = mybir.dt.float32
I32 = mybir.dt.int32

N_NODES = 100000
F_VIS = 512
F_LANG = 300
F_SPAT = 16
HID = 256
NCLS = 13
N_CORES = 8

ROW = 832                 # combined node row (bf16 elems): 512+300+1+19
ONE_COL = F_VIS + F_LANG  # 812: constant-1.0 column (bias trick)
FEAT = 2 * ROW            # 1664 = 13 * 128
NCHUNK = FEAT // 128      # 13
ES_COL = ROW + ONE_COL    # 1644: es destination (src-side pad region)
ES_W = FEAT - ES_COL      # 20: es region width (16 real + 4 zero pad)

TILE_E = 128              # edges per tile (PE partition dim)
G_TILES = 4               # tiles per group (batched gather/transpose)
GE = TILE_E * G_TILES     # 512 edges per group

E_FULL = 200000
E_PER = 25088             # 196 tiles of 128 (25000 real + 88 pad)
N_GROUPS = E_PER // GE    # 49

bf = ml_dtypes.bfloat16


def build_nc(n_groups=N_GROUPS, es_split=True, ps_o_bufs=1, ps_tf_bufs=3, lookahead=0, slot_carrier=False):
    e_per = n_groups * GE
    n_tiles = n_groups * G_TILES

    nc = bacc.Bacc(None, target_bir_lowering=False, debug=False)
    table_d = nc.declare_dram_parameter("table", [N_NODES, ROW], BF16, isOutput=False)
    w1_d = nc.declare_dram_parameter("w1", [FEAT, HID], BF16, isOutput=False)
    w2_d = nc.declare_dram_parameter("w2", [HID, NCLS], BF16, isOutput=False)
    b2_d = nc.declare_dram_parameter("b2", [1, NCLS], BF16, isOutput=False)
    didx_d = nc.declare_dram_parameter("didx", [TILE_E, n_tiles], I32, isOutput=False)
    sidx_d = nc.declare_dram_parameter("sidx", [TILE_E, n_tiles], I32, isOutput=False)
    es_d = nc.declare_dram_parameter(
        "es", [n_groups, TILE_E, G_TILES, ES_W], BF16, isOutput=False
    )
    out_d = nc.declare_dram_parameter("out", [e_per, NCLS], F32, isOutput=True)

    with TileContext(nc) as tc:
        with (
            tc.tile_pool(name="const", bufs=1) as constp,
            tc.tile_pool(name="gath", bufs=5) as gathp,
            tc.tile_pool(name="ft", bufs=5) as ftp,
            tc.tile_pool(name="hh", bufs=4) as hp,
            tc.tile_pool(name="ht", bufs=4) as htp,
            tc.tile_pool(name="oo", bufs=2) as outp,
            tc.tile_pool(name="ps_h", bufs=2, space="PSUM") as psh,
            tc.tile_pool(name="ps_tf", bufs=ps_tf_bufs, space="PSUM") as pstf,
            tc.tile_pool(name="ps_ht", bufs=2, space="PSUM") as psht,
            tc.tile_pool(name="ps_o", bufs=ps_o_bufs, space="PSUM") as pso,
        ):
            w1_sb = constp.tile([128, NCHUNK, HID], BF16)
            nc.sync.dma_start(
                out=w1_sb[:], in_=w1_d[:].rearrange("(c k) h -> k c h", k=128)
            )
            w2_sb = constp.tile([128, 2, NCLS], BF16)
            nc.sync.dma_start(
                out=w2_sb[:], in_=w2_d[:].rearrange("(c k) n -> k c n", k=128)
            )
            b2_sb = constp.tile([1, NCLS], BF16)
            nc.sync.dma_start(out=b2_sb[:], in_=b2_d[:])
            ones_sb = constp.tile([1, 128], BF16)
            nc.gpsimd.memset(ones_sb[:], 1.0)
            ident_sb = constp.tile([128, 128], BF16)
            make_identity(nc, ident_sb[:])
            # b2 broadcast across partitions (ones ⊗ b2) once, so the
            # per-tile bias add fuses into the DVE output copy
            b2b_ps = psht.tile([128, NCLS], F32, tag="psht")
            nc.tensor.matmul(
                out=b2b_ps[:], lhsT=ones_sb[:], rhs=b2_sb[:], start=True, stop=True
            )
            b2_bcast = constp.tile([128, NCLS], F32)
            nc.vector.tensor_copy(out=b2_bcast[:], in_=b2b_ps[:])
            didx_sb = constp.tile([TILE_E, n_tiles], I32)
            nc.sync.dma_start(out=didx_sb[:], in_=didx_d[:])
            sidx_sb = constp.tile([TILE_E, n_tiles], I32)
            nc.sync.dma_start(out=sidx_sb[:], in_=sidx_d[:])

            out_view = out_d[:].rearrange("(t p) c -> p t c", p=TILE_E)

            def emit_gathers(g):
                feat = gathp.tile([128, G_TILES, FEAT], BF16, tag="gath")
                if slot_carrier:
                    # tiny same-engine write absorbs the slot-release wait so
                    # the first gather doesn't need an event-semaphore split
                    nc.gpsimd.memset(feat[:1, 0, :2], 0.0)
                for t in range(G_TILES):
                    gt = g * G_TILES + t
                    nc.gpsimd.indirect_dma_start(
                        out=feat[:, t, 0:ROW],
                        out_offset=None,
                        in_=table_d[:],
                        in_offset=bass.IndirectOffsetOnAxis(
                            ap=didx_sb[:, gt : gt + 1], axis=0
                        ),
                    )
                    # src side gathers only row[0:812] (vis+lang) so the es
                    # region [1644:1664) has a single writer (the es DMA
                    # below) — keeps every gather at one semaphore wait
                    nc.gpsimd.indirect_dma_start(
                        out=feat[:, t, ROW : ES_COL if es_split else FEAT],
                        out_offset=None,
                        in_=table_d[:],
                        in_offset=bass.IndirectOffsetOnAxis(
                            ap=sidx_sb[:, gt : gt + 1], axis=0
                        ),
                    )
                # edge-spatial (padded to 20 cols with zeros on host) fills
                # the src-side tail [1644:1664) exactly
                nc.sync.dma_start(
                    out=feat[:, :, ES_COL:FEAT], in_=es_d[g, :, :, :]
                )
                return feat

            feats = {}

            def ensure_gathers(g):
                if g < n_groups and g not in feats:
                    feats[g] = emit_gathers(g)

            for g in range(n_groups):
                gsl = slice(g * G_TILES, (g + 1) * G_TILES)
                for k in range(g, g + 1 + lookahead):
                    ensure_gathers(k)
                feat = feats.pop(g)
                # feature-major chunks via PE transpose (the DMA xbar must
                # stay free: gathers are SDMA-descriptor-bound).  4 chunk
                # transposes land in one PSUM tile, copied out 512 wide;
                # copies alternate DVE/ACT to split the load.
                fview = feat[:].rearrange("p t f -> p (t f)")
                fT = ftp.tile([128, G_TILES * NCHUNK, 128], BF16, tag="ft")
                n_cg = G_TILES * NCHUNK // 4  # 13 copy-groups of 4 chunks
                for j in range(n_cg):
                    tp_ps = pstf.tile([128, 512], BF16, tag="pstf")
                    for jj in range(4):
                        c = 4 * j + jj
                        nc.tensor.transpose(
                            out=tp_ps[:, jj * 128 : (jj + 1) * 128],
                            in_=fview[:, c * 128 : (c + 1) * 128],
                            identity=ident_sb[:],
                        )
                    dst = fT[:, 4 * j : 4 * j + 4, :].rearrange("p c e -> p (c e)")
                    if j % 3 == 2:
                        nc.scalar.activation(
                            out=dst, in_=tp_ps[:],
                            func=mybir.ActivationFunctionType.Copy,
                        )
                    else:
                        nc.vector.tensor_copy(out=dst, in_=tp_ps[:])

                o_sb = outp.tile([128, G_TILES, NCLS], F32)
                for t in range(G_TILES):
                    h_ps = psh.tile([128, HID], F32)
                    for c in range(NCHUNK):
                        nc.tensor.matmul(
                            out=h_ps[:],
                            lhsT=fT[:, t * NCHUNK + c, :],
                            rhs=w1_sb[:, c, :],
                            start=(c == 0),
                            stop=(c == NCHUNK - 1),
                        )
                    h_sb = hp.tile([128, HID], BF16)
                    nc.scalar.activation(
                        out=h_sb[:], in_=h_ps[:], func=mybir.ActivationFunctionType.Relu
                    )
                    hT_ps = psht.tile([128, 256], BF16, tag="psht")
                    for i in range(2):
                        nc.tensor.transpose(
                            out=hT_ps[:, i * 128 : (i + 1) * 128],
                            in_=h_sb[:, i * 128 : (i + 1) * 128],
                            identity=ident_sb[:],
                        )
                    hT_sb = htp.tile([128, 2, 128], BF16)
                    nc.vector.tensor_copy(
                        out=hT_sb[:].rearrange("p c e -> p (c e)"), in_=hT_ps[:]
                    )
                    o_ps = pso.tile([128, NCLS], F32)
                    nc.tensor.matmul(
                        out=o_ps[:], lhsT=hT_sb[:, 0, :], rhs=w2_sb[:, 0, :],
                        start=True, stop=False,
                    )
                    nc.tensor.matmul(
                        out=o_ps[:], lhsT=hT_sb[:, 1, :], rhs=w2_sb[:, 1, :],
                        start=False, stop=True,
                    )
                    nc.vector.tensor_add(
                        out=o_sb[:, t, :], in0=o_ps[:], in1=b2_bcast[:]
                    )
                nc.sync.dma_start(out=out_view[:, gsl, :], in_=o_sb[:])
    nc.finalize()
    return nc


def prep_shared(node_feat, node_feat_lang, W1, b1, W2, b2):
    """Host-side packing of the replicated (per-core-identical) inputs."""
    table = np.zeros((N_NODES, ROW), dtype=bf)
    table[:, :F_VIS] = node_feat.astype(bf)
    table[:, F_VIS:ONE_COL] = node_feat_lang.astype(bf)
    table[:, ONE_COL] = bf(1.0)

    # Reference concat order: [dst_vis, dst_lang, s_f, src_lang, src_vis]
    w1p = np.zeros((FEAT, HID), np.float32)
    w1p[0:F_VIS] = W1[0:F_VIS]                            # dst vis
    w1p[F_VIS:ONE_COL] = W1[F_VIS:ONE_COL]                # dst lang
    w1p[ONE_COL] = b1                                     # bias row (dst 1.0 col)
    w1p[ROW : ROW + F_VIS] = W1[ONE_COL + F_SPAT + F_LANG :]   # src vis
    w1p[ROW + F_VIS : ROW + ONE_COL] = W1[ONE_COL + F_SPAT : ONE_COL + F_SPAT + F_LANG]  # src lang
    w1p[ES_COL : ES_COL + F_SPAT] = W1[ONE_COL : ONE_COL + F_SPAT]  # s_f rows

    return {
        "table": table,
        "w1": w1p.astype(bf),
        "w2": W2.astype(bf),
        "b2": b2.reshape(1, NCLS).astype(bf),
    }


def prep_core(didx, sidx, es, n_groups=N_GROUPS):
    """Host-side packing of one core's edge shard (padding + layout)."""
    e_per = n_groups * GE
    n_tiles = n_groups * G_TILES
    n = didx.shape[0]

    dpad = np.zeros(e_per, np.int32)
    spad = np.zeros(e_per, np.int32)
    dpad[:n] = didx
    spad[:n] = sidx
    espad = np.zeros((e_per, ES_W), np.float32)
    espad[:n, :F_SPAT] = es

    return {
        "didx": np.ascontiguousarray(dpad.reshape(n_tiles, TILE_E).T),
        "sidx": np.ascontiguousarray(spad.reshape(n_tiles, TILE_E).T),
        # [g, p, t, 16]: es[g, p, t] = edge g*512 + t*128 + p
        "es": np.ascontiguousarray(
            espad.reshape(n_groups, G_TILES, TILE_E, ES_W).transpose(0, 2, 1, 3)
        ).astype(bf),
    }


_NC_CACHE = {}


def _get_nc(n_groups=N_GROUPS):
    if n_groups not in _NC_CACHE:
        _NC_CACHE[n_groups] = build_nc(n_groups)
    return _NC_CACHE[n_groups]


def _install_trace_shim():
    """Enable NTFF profiling under axon when the image's antenv lacks
    axon_hooks: register a minimal hook registry + the ctypes-driven
    profile hook, and neuter the artifact upload."""
    import sys
    import types

    try:
        import antenv
        from trn_agent_boot.trn_boot import _ntff_profile_via_ctypes

        if "antenv.axon_hooks" not in sys.modules:
            mod = types.ModuleType("antenv.axon_hooks")
            mod._hook = None

            def set_axon_ntff_profile_hook(h):
                mod._hook = h

            def get_axon_ntff_profile_hook():
                return mod._hook

            mod.set_axon_ntff_profile_hook = set_axon_ntff_profile_hook
            mod.get_axon_ntff_profile_hook = get_axon_ntff_profile_hook
            sys.modules["antenv.axon_hooks"] = mod
            antenv.axon_hooks = mod
        hooks = sys.modules["antenv.axon_hooks"]
        if hooks.get_axon_ntff_profile_hook() is None:
            hooks.set_axon_ntff_profile_hook(
                _ntff_profile_via_ctypes("/opt/axon/libaxon_pjrt.so")
            )

        import concourse.bass_utils as bu

        bu.upload_artifacts = lambda tmpdir: f"local:{tmpdir}"
        return True
    except Exception as e:  # degrade silently — tracing is optional
        print(f"trace shim unavailable: {type(e).__name__}: {e}")
        return False


last_exec_time_ns = None
last_results = None


def kernel(**inputs):
    global last_exec_time_ns, last_results
    from concourse.bass_utils import run_bass_kernel_spmd

    node_feat = np.asarray(inputs["node_feat"], np.float32)
    node_feat_lang = np.asarray(inputs["node_feat_lang"], np.float32)
    edge_spatial = np.asarray(inputs["edge_spatial"], np.float32)
    W1 = np.asarray(inputs["W1"], np.float32)
    b1 = np.asarray(inputs["b1"], np.float32)
    W2 = np.asarray(inputs["W2"], np.float32)
    b2 = np.asarray(inputs["b2"], np.float32)
    src_idx = np.asarray(inputs["src_idx"]).astype(np.int32)
    dst_idx = np.asarray(inputs["dst_idx"]).astype(np.int32)

    E = dst_idx.shape[0]
    e_core = (E + N_CORES - 1) // N_CORES  # 25000

    shared = prep_shared(node_feat, node_feat_lang, W1, b1, W2, b2)

    in_maps = []
    for c in range(N_CORES):
        lo, hi = c * e_core, min((c + 1) * e_core, E)
        m = dict(shared)
        m.update(prep_core(dst_idx[lo:hi], src_idx[lo:hi], edge_spatial[lo:hi]))
        in_maps.append(m)

    nc = _get_nc()
    trace = os.environ.get("KERNEL_TRACE", "0") == "1"
    if trace:
        _install_trace_shim()
    res = run_bass_kernel_spmd(
        nc, in_maps, core_ids=list(range(N_CORES)), trace=trace
    )
    last_exec_time_ns = res.exec_time_ns
    last_results = res

    out = np.empty((E, NCLS), np.float32)
    for c in range(N_CORES):
        lo, hi = c * e_core, min((c + 1) * e_core, E)
        out[lo:hi] = res.results[c]["out"][: hi - lo]
    return out

